# revision 9
# baseline (speedup 1.0000x reference)
"""KMISPool kernel for Trainium2 (8 NeuronCores, raw Bass SPMD).

Host: graph preprocessing + k-MIS int32 fixed point (~1.5 s numpy) + output
assembly.  Device (8 cores, node/edge sharded) does the memory-bound math:
  (a) x_pooled = onehot(cluster)^T @ x      -- 102.4 MB feature reads; DVE
      builds one-hot selection tiles, PE accumulates into PSUM.
  (b) prefix sums of key-sorted edge_attr   -- DVE scan; host turns prefix
      differences at segment boundaries into the coalesced pooled_edge_attr.

Raw Bass with explicit semaphores; every instruction carries at most ONE
sync wait (this toolchain's walrus rejects multi-wait instructions).
"""

from contextlib import ExitStack

import numpy as np

N = 100_000
E = 3_200_000
D = 256
K = 2
NCORES = 8
P = 128
CBLK = 768                  # cluster one-hot width (c = 713 for this input)
NBLK = CBLK // P            # 6
NCH = 98                    # node chunks/core: 12500 -> 98*128 padded
NPAD = NCH * P
EPC = E // NCORES           # 400000 edges/core
EROW = EPC // P             # 3125 edges per partition row
XB = 4                      # x-chunk double buffers
OB = 4                      # one-hot buffers


LAST_SPMD_WALL_S = 0.0


def _seg_min(vals, seg, n):
    out = np.full(n, np.iinfo(np.int64).max, dtype=np.int64)
    np.minimum.at(out, seg, vals)
    return out


def _seg_max(vals, seg, n):
    out = np.zeros(n, dtype=np.int64)
    np.maximum.at(out, seg, vals)
    return out


def _host_mis_cluster(row, col, n):
    rank = np.arange(n, dtype=np.int64)
    mis = np.zeros(n, dtype=bool)
    mask = np.zeros(n, dtype=bool)
    min_rank = rank.copy()
    while not mask.all():
        for _ in range(K):
            min_rank = np.minimum(min_rank, _seg_min(min_rank[row], col, n))
        mis |= rank == min_rank
        mm = mis.copy()
        for _ in range(K):
            mm |= _seg_max(mm[row].astype(np.int64), col, n) > 0
        mask = mm
        min_rank = np.where(mask, n, rank)
    min_rank = np.where(mis, rank, n)
    for _ in range(K):
        min_rank = np.minimum(min_rank, _seg_min(min_rank[row], col, n))
    _, clusters = np.unique(min_rank, return_inverse=True)
    return mis, clusters.astype(np.int64)


def _build_device_kernel():
    import concourse.bass as bass
    import concourse.mybir as mybir

    f32 = mybir.dt.float32
    Alu = mybir.AluOpType
    nc = bass.Bass()

    xc = nc.dram_tensor("xc", [NPAD, D], f32, kind="ExternalInput")
    cluT = nc.dram_tensor("cluT", [P, NCH], f32, kind="ExternalInput")
    iota = nc.dram_tensor("iota", [P, CBLK], f32, kind="ExternalInput")
    attrS = nc.dram_tensor("attrS", [P, EROW], f32, kind="ExternalInput")
    xp = nc.dram_tensor("xp", [CBLK, D], f32, kind="ExternalOutput")
    segp = nc.dram_tensor("segp", [P, EROW], f32, kind="ExternalOutput")

    with ExitStack() as ctx:
        sb = lambda name, shape: ctx.enter_context(
            nc.sbuf_tensor(name, shape, f32)
        )
        iota_t = sb("iota_t", [P, CBLK])
        cluT_t = sb("cluT_t", [P, NCH])
        attr_t = sb("attr_t", [P, EROW])
        scan_t = sb("scan_t", [P, EROW])
        xt = [sb(f"xt{i}", [P, D]) for i in range(XB)]
        oh = [sb(f"oh{i}", [P, CBLK]) for i in range(OB)]
        ot = sb("ot", [P, NBLK * D])
        ps = [
            ctx.enter_context(nc.psum_tensor(f"ps{b}", [P, D], f32))
            for b in range(NBLK)
        ]
        sconst = ctx.enter_context(nc.semaphore("sconst"))
        sxt = [
            ctx.enter_context(nc.semaphore(f"sxt{i}")) for i in range(XB)
        ]
        sdve = ctx.enter_context(nc.semaphore("sdve"))
        spe = ctx.enter_context(nc.semaphore("spe"))
        sout = ctx.enter_context(nc.semaphore("sout"))

        # constant loads (own semaphore: completions across queues are
        # unordered relative to the xt stream below)
        nc.sync.dma_start(iota_t[:], iota[:]).then_inc(sconst, 16)
        nc.sync.dma_start(cluT_t[:], cluT[:]).then_inc(sconst, 16)
        nc.sync.dma_start(attr_t[:], attrS[:]).then_inc(sconst, 16)

        # (b) per-partition-row prefix sums of edge_attr
        nc.vector.wait_ge(sconst, 48)
        nc.vector.tensor_tensor_scan(
            out=scan_t[:],
            data0=attr_t[:],
            data1=attr_t[:],
            initial=0.0,
            op0=Alu.add,
            op1=Alu.bypass,
        ).then_inc(sdve, 1)

        # (a) pooled features
        for ch in range(NCH):
            if ch >= XB:
                nc.sync.wait_ge(spe, NBLK * (ch - XB + 1))
            nc.sync.dma_start(
                xt[ch % XB][:], xc[ch * P : (ch + 1) * P, :]
            ).then_inc(sxt[ch % XB], 16)

            nc.vector.wait_ge(sxt[ch % XB], 16 * (ch // XB + 1))
            if ch >= OB:
                nc.vector.wait_ge(spe, NBLK * (ch - OB + 1))
            nc.vector.tensor_tensor(
                out=oh[ch % OB][:],
                in0=cluT_t[:, ch : ch + 1].to_broadcast([P, CBLK]),
                in1=iota_t[:],
                op=Alu.is_equal,
            ).then_inc(sdve, 1)

            nc.tensor.wait_ge(sdve, 2 + ch)
            for b in range(NBLK):
                nc.tensor.matmul(
                    ps[b][:],
                    lhsT=oh[ch % OB][:, b * P : (b + 1) * P],
                    rhs=xt[ch % XB][:],
                    start=(ch == 0),
                    stop=(ch == NCH - 1),
                ).then_inc(spe, 1)

        # evacuate PSUM, write outputs
        nc.vector.wait_ge(spe, NBLK * NCH)
        for b in range(NBLK):
            nc.vector.tensor_copy(
                out=ot[:, b * D : (b + 1) * D], in_=ps[b][:]
            ).then_inc(sdve, 1)

        nc.sync.wait_ge(sdve, 1)
        nc.sync.dma_start(segp[:], scan_t[:]).then_inc(sout, 16)
        nc.sync.wait_ge(sdve, 1 + NCH + NBLK)
        nc.sync.dma_start(
            xp[:].rearrange("(b p) d -> p b d", p=P),
            ot[:].rearrange("p (b d) -> p b d", b=NBLK),
        ).then_inc(sout, 16)

        for eng in nc.engines.values():
            eng.wait_ge(sout, 32)
    return nc


def kernel(x, edge_index, edge_attr):
    from concourse.bass_utils import run_bass_kernel_spmd

    x = np.asarray(x, dtype=np.float32)
    edge_index = np.asarray(edge_index)
    edge_attr = np.asarray(edge_attr, dtype=np.float32)
    n = x.shape[0]
    row = edge_index[0].astype(np.int64)
    col = edge_index[1].astype(np.int64)

    mis, cluster = _host_mis_cluster(row, col, n)
    c = int(mis.sum())
    assert c <= CBLK - 16, c

    keys = cluster[row] * c + cluster[col]
    uniq, counts = np.unique(keys, return_counts=True)
    perm = np.argsort(keys, kind="stable")
    attr_sorted = edge_attr[perm]

    n_per_core = n // NCORES
    iota_host = np.broadcast_to(
        np.arange(CBLK, dtype=np.float32), (P, CBLK)
    ).copy()

    in_maps = []
    for ci in range(NCORES):
        ns = ci * n_per_core
        xcore = np.zeros((NPAD, D), dtype=np.float32)
        xcore[:n_per_core] = x[ns : ns + n_per_core]
        clu = np.full(NPAD, CBLK - 8, dtype=np.float32)
        clu[:n_per_core] = cluster[ns : ns + n_per_core].astype(np.float32)
        cluT_h = clu.reshape(NCH, P).T.copy()
        es = ci * EPC
        attr_rows = attr_sorted[es : es + EPC].reshape(P, EROW).copy()
        in_maps.append(
            {"xc": xcore, "cluT": cluT_h, "iota": iota_host, "attrS": attr_rows}
        )

    nc = _build_device_kernel()
    import time as _time

    _t0 = _time.time()
    res = run_bass_kernel_spmd(nc, in_maps, core_ids=list(range(NCORES)))
    global LAST_SPMD_WALL_S
    LAST_SPMD_WALL_S = _time.time() - _t0

    # ---- host reduce / boundary extraction ----
    xp = np.zeros((CBLK, D), dtype=np.float32)
    prefix = np.empty((NCORES * P, EROW), dtype=np.float64)
    for ci in range(NCORES):
        xp += res.results[ci]["xp"]
        prefix[ci * P : (ci + 1) * P] = res.results[ci]["segp"]
    x_pooled = xp[:c]

    # global inclusive prefix over the sorted edge order
    row_tot = prefix[:, -1]
    g_prev = np.concatenate(([0.0], np.cumsum(row_tot)))[:-1]
    ends = np.cumsum(counts)            # exclusive segment ends
    starts = ends - counts

    def pg(e):                           # inclusive prefix at edge position e
        return g_prev[e // EROW] + prefix[e // EROW, e % EROW]

    val = pg(ends - 1) - np.where(starts > 0, pg(np.maximum(starts - 1, 0)), 0.0)
    val = val.astype(np.float32)

    new_row = (uniq // c).astype(np.int32)
    new_col = (uniq % c).astype(np.int32)
    keep = np.nonzero(new_row != new_col)[0]
    pooled_edge_index = np.stack([new_row[keep], new_col[keep]]).astype(np.int32)
    pooled_edge_attr = val[keep]
    return x_pooled, pooled_edge_index, pooled_edge_attr
